# revision 2
# baseline (speedup 1.0000x reference)
"""Trainium2 Bass kernel for llama-style GQA causal attention (B=4, S=1024, D=4096,
32 Q heads / 8 KV heads, head_dim=128, RoPE) — all-bf16 restructure.

Sharding: 8 cores = 4 batches x 2 head-halves (tensor-parallel over heads).
Core c handles batch b=c//2 and head-half g=c%2 (16 Q heads, 4 KV heads).
Each core computes a partial y^T = (attn_heads @ wo_half)^T in [D, S] layout;
the host sums the two head-half partials per batch and transposes back.

Structure:
  - all matmul operands bf16 (same PE rate as fp32r at wide tiles, no 4x
    narrow-tile penalty, 2x DVE on SBUF elementwise, half DMA/SBUF bytes).
  - phase 1: single x^T super-block, q/k/v accumulate fully in PSUM;
    roped q/k stay in SBUF (no DRAM spill).
  - phase 2: key-tiles processed in (narrow, full) pairs sharing a
    [128, 1024] PSUM tile -> ONE exp per pair (ACT fixed-cost amortized);
    flat pair stream software-pipelined so exp+mask latency hides under
    scores matmuls.  Causal mask = 0/1 multiply on the exp output.
  - single PSUM pool, tags ping-ponged across groups (no pool barriers);
    phase-3 wo tiles prefetch during phase 2 automatically.
  - paired PSUM evictions ([128,1024] ACT copies, single y DMAs).
"""

import numpy as np

import concourse.bacc as bacc
import concourse.mybir as mybir
import concourse.tile as tile
from concourse.bass_utils import run_bass_kernel_spmd

# problem shape (hardcoded per contract)
B, S, D = 4, 1024, 4096
NH, NKV, HD = 32, 8, 128
P = 128
G2 = 2                      # head-halves (TP degree per batch)
QH = NH // G2               # 16 q heads per core
KVH = NKV // G2             # 4 kv heads per core
QD, KVD = QH * HD, KVH * HD # 2048, 512
THETA = 10000.0
SCALE = float(1.0 / np.sqrt(HD))

NKT = D // P                # 32 k-tiles over the model dim
TC = 512                    # token chunk (matmul free dim)
NTC = S // TC               # 2
NTOK = S // P               # 8 token tiles

F32 = mybir.dt.float32
BF16 = mybir.dt.bfloat16

_CACHE = {}


def _body(nc, tc_, io):
    xt, wq, wk, wv, wo, swp, cosf, sinf, mask01, ones, yt = io
    ts = lambda i, n: slice(i * n, (i + 1) * n)

    with (
        tc_.tile_pool(name="cp", bufs=1) as cp,
        tc_.tile_pool(name="wrk", bufs=1) as wrk,
        tc_.tile_pool(name="ps", bufs=1, space="PSUM") as psp,
    ):
        acc_k = cp.tile([P, KVH, S], BF16)
        acc_v = cp.tile([P, NTOK, KVD], BF16)
        acc_q = cp.tile([P, QH, S], BF16)
        acc_o = cp.tile([P, QH, S], BF16)
        xtb = cp.tile([P, NKT, S], BF16)

        swp_sb = cp.tile([P, P], BF16)
        mask_sb = cp.tile([P, P], BF16)
        ones_sb = cp.tile([P, 1], BF16)
        cos_sb = cp.tile([P, S], BF16)
        sin_sb = cp.tile([P, S], BF16)

        # PSUM budget (16KB/partition): a = 2 x [128,1024] pair slots (8KB),
        # o/c = 2 x [128,512] single slots each (4KB + 4KB).  Projection /
        # wo groups ping-pong between the a-pairs and the o+c singles so
        # consecutive groups never share banks.
        def ps_pair(name):
            return psp.tile([P, 2 * TC], F32, tag="a", name=name, bufs=2)

        def ps_single(name, tg):
            return psp.tile([P, TC], F32, tag=tg, name=name, bufs=2)

        # ---------------- phase 1: projections + rope ----------------
        xt_r = xt.ap().rearrange("(kt p) t -> p kt t", p=P)
        wq_r = wq.ap().rearrange("(kt p) m -> kt p m", p=P)
        wk_r = wk.ap().rearrange("(kt p) m -> kt p m", p=P)
        wv_r = wv.ap().rearrange("(kt p) m -> kt p m", p=P)

        # first k-weight tile ahead of everything on the sync queue, then
        # x^T tile 0; remaining x tiles stream on the scalar queue just
        # ahead of their consumption.
        w0_t = wrk.tile([P, KVD], BF16, tag="w", name="w0_t", bufs=6)
        w0_t = w0_t[:, :2 * P]
        nc.sync.dma_start(w0_t, wk_r[0, :, ts(0, 2 * P)])
        nc.sync.dma_start(xtb[:, 0, :TC], xt_r[:, 0, :TC])
        nc.sync.dma_start(xtb[:, 0, TC:], xt_r[:, 0, TC:])
        for kt in range(1, NKT):
            nc.scalar.dma_start(xtb[:, kt], xt_r[:, kt])
            if kt == 2:
                nc.scalar.dma_start(swp_sb, swp.ap())
                nc.scalar.dma_start(cos_sb, cosf.ap())
                nc.scalar.dma_start(sin_sb, sinf.ap())
            if kt == 8:
                nc.scalar.dma_start(mask_sb, mask01.ap())
                nc.scalar.dma_start(ones_sb, ones.ap())

        def rope(A, h):
            # in-place rope over A[:, h, :]: A = A*cos + swap(A)*sin
            for t in range(NTC):
                src = A[:, h, ts(t, TC)]
                ps_sw = ps_single("ps_sw", "c")
                nc.tensor.matmul(ps_sw, swp_sb, src, start=True, stop=True)
                tmp = wrk.tile([P, TC], BF16, tag="ropet", name="ropet", bufs=4)
                nc.vector.tensor_mul(tmp, ps_sw, sin_sb[:, ts(t, TC)])
                qr = wrk.tile([P, TC], BF16, tag="ropeq", name="ropeq", bufs=4)
                nc.gpsimd.tensor_mul(qr, src, cos_sb[:, ts(t, TC)])
                nc.gpsimd.tensor_add(src, qr, tmp)

        def group_tiles(gi, names):
            # ping-pong: even groups use the two [128,1024] a-pairs as 4
            # halves, odd groups use the 4 o/c singles.
            if gi % 2 == 0:
                pA = ps_pair(names + "A")
                pB = ps_pair(names + "B")
                halves = [pA[:, :TC], pA[:, TC:], pB[:, :TC], pB[:, TC:]]
                return halves, [(pA, 0, 2), (pB, 2, 2)]
            sing = [ps_single(names + str(_i), "oocc"[_i]) for _i in range(4)]
            return sing, [(sing[_i], _i, 1) for _i in range(4)]

        _gi = {"n": 0}
        _pending_ropes = []

        def flush_ropes():
            while _pending_ropes:
                A, h = _pending_ropes.pop(0)
                rope(A, h)

        def proj_mgroup(w_r, A, mg, w0=None, evict_dve=False):
            # one group: 2 m-subtiles x 2 token-chunks, full 32-kt psum accum
            # tile order: [i0t0, i0t1, i1t0, i1t1]
            gi = _gi["n"]; _gi["n"] += 1
            ps, evs = group_tiles(gi, "psg")
            for kt in range(NKT):
                if kt == 0 and w0 is not None:
                    w_t = w0
                else:
                    w_t = wrk.tile([P, KVD], BF16, tag="w", name="w_t", bufs=6)
                    w_t = w_t[:, :2 * P]
                    nc.sync.dma_start(w_t, w_r[kt, :, ts(mg, 2 * P)])
                for i in range(2):
                    for t in range(NTC):
                        nc.tensor.matmul(
                            ps[2 * i + t], w_t[:, ts(i, P)],
                            xtb[:, kt, ts(t, TC)],
                            start=(kt == 0), stop=(kt == NKT - 1))
                if kt == 5:
                    # previous group's evictions have landed by now; its rope
                    # swap matmuls slot in without stalling on the ACT copies
                    flush_ropes()
            # evict: A[:, mg*2+i, :] rows are contiguous [t0|t1]
            for ps_t, base, ntile in evs:
                i0 = base // 2
                dst = (A[:, mg * 2 + i0, base % 2 * TC:(base % 2 + ntile) * TC]
                       if ntile == 1 else A[:, mg * 2 + i0, :])
                if evict_dve:
                    nc.vector.tensor_copy(dst, ps_t)
                else:
                    nc.scalar.activation(
                        dst, ps_t, mybir.ActivationFunctionType.Copy)
            _pending_ropes.append((A, mg * 2))
            _pending_ropes.append((A, mg * 2 + 1))

        # k first (rope early), then v, then q
        for mg in range(KVH // 2):
            proj_mgroup(wk_r, acc_k, mg, w0=w0_t if mg == 0 else None)

        for tg in range(NTOK // 4):
            gi = _gi["n"]; _gi["n"] += 1
            ps, evs = group_tiles(gi, "psv")
            for kt in range(NKT):
                w_t = wrk.tile([P, KVD], BF16, tag="w", name="w_tv", bufs=6)
                nc.sync.dma_start(w_t, wv_r[kt])
                for tm in range(4):
                    nc.tensor.matmul(
                        ps[tm], xtb[:, kt, ts(tg * 4 + tm, P)], w_t,
                        start=(kt == 0), stop=(kt == NKT - 1))
                if kt == 5:
                    flush_ropes()
            for ps_t, base, ntile in evs:
                nc.scalar.activation(
                    acc_v[:, tg * 4 + base:tg * 4 + base + ntile],
                    ps_t, mybir.ActivationFunctionType.Copy)

        for mg in range(QH // 2):
            # last group evicts on DVE so ACT is free for phase 2's first exp
            proj_mgroup(wq_r, acc_q, mg, evict_dve=(mg == QH // 2 - 1))
        # the last group's ropes (heads 14/15) are flushed a few steps into
        # phase 2 — their swap matmuls would otherwise stall the in-order PE
        # right at the boundary while blocking scores that don't need them.

        # ---------------- phase 2: attention (paired, software-pipelined) ----
        # per chunk (h, t): key-tiles are processed as (narrow, full) pairs
        # sharing one [128, 2*TC] psum tile; one exp covers the contiguous
        # union.  lo (ones+PV accumulation) consumes pairs LAG steps later.
        def kc_off(kc, t):
            return max(0, kc - 4 * t) * P

        PAIRS = {0: [(3, 0), (2, 1)],
                 1: [(7, 0), (6, 1), (5, 2), (4, 3)]}

        chunks = [(h, t) for h in range(QH) for t in range(NTC)]
        steps = []
        cstate = {}
        for ci, (h, t) in enumerate(chunks):
            for pi in range(len(PAIRS[t])):
                steps.append((ci, pi))

        def emit_scores(ci, pi):
            h, t = chunks[ci]
            g = h // (QH // KVH)
            kc_a, kc_b = PAIRS[t][pi]
            off_a = kc_off(kc_a, t)
            ps2 = ps_pair("ps2")
            nc.tensor.matmul(ps2[:, off_a:TC], acc_k[:, g, ts(kc_a, P)],
                             acc_q[:, h, t * TC + off_a:(t + 1) * TC],
                             start=True, stop=True)
            # right member written full-width so the exp range is contiguous
            nc.tensor.matmul(ps2[:, TC:], acc_k[:, g, ts(kc_b, P)],
                             acc_q[:, h, ts(t, TC)],
                             start=True, stop=True)
            p2 = wrk.tile([P, 2 * TC], BF16, tag="p", name="p2", bufs=4)
            nc.scalar.activation(p2[:, off_a:], ps2[:, off_a:],
                                 mybir.ActivationFunctionType.Exp,
                                 scale=SCALE)
            for kc, base in ((kc_a, 0), (kc_b, TC)):
                j = kc - 4 * t
                if j >= 0:
                    off = base + kc_off(kc, t)
                    nc.vector.tensor_mul(p2[:, off:off + P],
                                         p2[:, off:off + P], mask_sb)
            cstate[(ci, pi)] = p2

        def emit_lo(ci, pi):
            h, t = chunks[ci]
            g = h // (QH // KVH)
            npair = len(PAIRS[t])
            if pi == 0:
                cstate[ci] = (ps_single("ps_o", "o"),
                              ps_single("ps_l", "c")[:1])
            ps_o, ps_l = cstate[ci]
            p2 = cstate.pop((ci, pi))
            kc_a, kc_b = PAIRS[t][pi]
            # full member first so the psum group starts full-width
            for kc, base in ((kc_b, TC), (kc_a, 0)):
                off = kc_off(kc, t)
                first = (pi == 0 and base == TC)
                last = (pi == npair - 1 and base == 0)
                nc.tensor.matmul(ps_l[:, off:], ones_sb,
                                 p2[:, base + off:base + TC],
                                 start=first, stop=last,
                                 skip_group_check=True)
                nc.tensor.matmul(ps_o[:, off:], acc_v[:, kc, ts(g, P)],
                                 p2[:, base + off:base + TC],
                                 start=first, stop=last,
                                 skip_group_check=True)
            if pi == npair - 1:
                del cstate[ci]
                rl = wrk.tile([1, TC], F32, tag="rl", name="rl", bufs=4)
                nc.vector.reciprocal(rl, ps_l)
                rlb = wrk.tile([P, TC], F32, tag="rlb", name="rlb", bufs=4)
                nc.gpsimd.partition_broadcast(rlb, rl)
                nc.vector.tensor_mul(acc_o[:, h, ts(t, TC)], ps_o, rlb)

        LAG = 2
        for i, (ci, pi) in enumerate(steps):
            emit_scores(ci, pi)
            if i == 6:
                flush_ropes()
            if i >= LAG:
                emit_lo(*steps[i - LAG])
        for i in range(len(steps) - LAG, len(steps)):
            emit_lo(*steps[i])

        # ---------------- phase 3: wo ----------------
        wo_r = wo.ap().rearrange("(kt p) m -> kt p m", p=P)
        NYG = D // (2 * P)
        _gi["n"] += 1   # align parity so the last yg gets pair tiles
        for yg in range(NYG):
            gi = _gi["n"]; _gi["n"] += 1
            ps, evs = group_tiles(gi, "psy")
            for kt in range(QD // P):
                w_t = wrk.tile([P, KVD], BF16, tag="w", name="w_to", bufs=6)
                w_t = w_t[:, :2 * P]
                nc.sync.dma_start(w_t, wo_r[kt, :, ts(yg, 2 * P)])
                for i in range(2):
                    for t in range(NTC):
                        nc.tensor.matmul(
                            ps[2 * i + t], w_t[:, ts(i, P)],
                            acc_o[:, kt, ts(t, TC)],
                            start=(kt == 0), stop=(kt == QD // P - 1))
            # evictions: pair tiles cover a full [128, S] output row-block
            n_ev = 0
            for ps_t, base, ntile in evs:
                i0 = base // 2
                mt = yg * 2 + i0
                if ntile == 2:
                    y_sb = wrk.tile([P, 2 * TC], BF16, tag="ysb", name="y_sb",
                                    bufs=4)
                    if yg == NYG - 1 and base == 2:
                        nc.vector.tensor_copy(y_sb, ps_t)
                    else:
                        nc.scalar.activation(
                            y_sb, ps_t, mybir.ActivationFunctionType.Copy)
                    eng = nc.scalar if (yg + n_ev) % 2 == 0 else nc.sync
                    eng.dma_start(yt.ap()[ts(mt, P), :], y_sb)
                else:
                    t = base % 2
                    y_sb = wrk.tile([P, 2 * TC], BF16, tag="ysb", name="y_sbs",
                                    bufs=4)
                    y_sb = y_sb[:, :TC]
                    nc.scalar.activation(
                        y_sb, ps_t, mybir.ActivationFunctionType.Copy)
                    eng = nc.scalar if (yg + n_ev) % 2 == 0 else nc.sync
                    eng.dma_start(yt.ap()[ts(mt, P), ts(t, TC)], y_sb)
                n_ev += 1


def _build(loop_k=0):
    nc = bacc.Bacc("TRN2", target_bir_lowering=False, debug=False)
    xt = nc.dram_tensor("xt", [D, S], BF16, kind="ExternalInput")
    wq = nc.dram_tensor("wq", [D, QD], BF16, kind="ExternalInput")
    wk = nc.dram_tensor("wk", [D, KVD], BF16, kind="ExternalInput")
    wv = nc.dram_tensor("wv", [D, KVD], BF16, kind="ExternalInput")
    wo = nc.dram_tensor("wo", [QD, D], BF16, kind="ExternalInput")
    swp = nc.dram_tensor("swp", [P, P], BF16, kind="ExternalInput")
    cosf = nc.dram_tensor("cosf", [P, S], BF16, kind="ExternalInput")
    sinf = nc.dram_tensor("sinf", [P, S], BF16, kind="ExternalInput")
    mask01 = nc.dram_tensor("mask01", [P, P], BF16, kind="ExternalInput")
    ones = nc.dram_tensor("ones", [P, 1], BF16, kind="ExternalInput")
    yt = nc.dram_tensor("yt", [D, S], BF16, kind="ExternalOutput")

    io = (xt, wq, wk, wv, wo, swp, cosf, sinf, mask01, ones, yt)
    with tile.TileContext(nc) as tc_:
        if loop_k:
            with tc_.For_i(0, loop_k, 1):
                _body(nc, tc_, io)
        else:
            _body(nc, tc_, io)
    nc.compile()
    return nc


def get_nc():
    if "nc" not in _CACHE:
        _CACHE["nc"] = _build()
    return _CACHE["nc"]


def host_inputs(x, wq, wk, wv, wo):
    """Shard + lay out the full inputs into per-core in_maps (bf16)."""
    import ml_dtypes
    bf = ml_dtypes.bfloat16
    x = np.asarray(x, np.float32)
    wq = np.asarray(wq, np.float32)
    wk = np.asarray(wk, np.float32)
    wv = np.asarray(wv, np.float32)
    wo = np.asarray(wo, np.float32)

    # rope tables in [hd, token] layout, pair-duplicated over partitions
    freqs = 1.0 / (THETA ** (np.arange(0, HD, 2, dtype=np.float32) / HD))
    ang = np.outer(np.arange(S, dtype=np.float32), freqs)  # [S, 64]
    cosf = np.repeat(np.cos(ang), 2, axis=1).T.astype(bf).copy()  # [128, S]
    sinf = np.repeat(np.sin(ang), 2, axis=1).T.astype(bf).copy()

    # pair-swap matrix (lhsT): matmul computes lhsT.T @ q = S_swap @ q
    sw = np.zeros((P, P), np.float32)
    for i in range(P // 2):
        sw[2 * i, 2 * i + 1] = -1.0
        sw[2 * i + 1, 2 * i] = 1.0
    swp = np.ascontiguousarray(sw.T).astype(bf)

    kp = np.arange(P)[:, None]
    qf = np.arange(P)[None, :]
    mask01 = np.where(kp <= qf, 1.0, 0.0).astype(bf)

    ones = np.ones((P, 1), bf)

    in_maps = []
    for c in range(8):
        b, g = c // G2, c % G2
        in_maps.append({
            "xt": np.ascontiguousarray(x[b].T).astype(bf),
            "wq": np.ascontiguousarray(wq[:, g * QD:(g + 1) * QD]).astype(bf),
            "wk": np.ascontiguousarray(wk[:, g * KVD:(g + 1) * KVD]).astype(bf),
            "wv": np.ascontiguousarray(wv[:, g * KVD:(g + 1) * KVD]).astype(bf),
            "wo": np.ascontiguousarray(wo[g * QD:(g + 1) * QD]).astype(bf),
            "swp": swp, "cosf": cosf, "sinf": sinf, "mask01": mask01,
            "ones": ones,
        })
    return in_maps


def kernel(x, wq, wk, wv, wo):
    in_maps = host_inputs(x, wq, wk, wv, wo)
    nc = get_nc()
    res = run_bass_kernel_spmd(nc, in_maps, core_ids=list(range(8)))
    y = np.empty((B, S, D), np.float32)
    for b in range(B):
        y[b] = (res.results[G2 * b]["yt"].astype(np.float32)
                + res.results[G2 * b + 1]["yt"].astype(np.float32)).T
    return y
